# revision 29
# baseline (speedup 1.0000x reference)
"""Trainium2 Bass kernel for nn_CustomLoss (CrossEntropy + binary-remap BCE).

loss = mean_i[ logsumexp(pred_i) - pred_i[t_i] ]
     + 100 * mean_i[ 1{ LUT[argmax(pred_i)] != LUT[t_i] } ]

with LUT = [0,0,1,1,1,1,1,1,0,0]  (LUT[j] = 1 iff 2 <= j <= 7).

Sharding: data-parallel over the batch axis across 8 NeuronCores; each core
returns per-partition partial sums which the host folds into the scalar.

Layout: the host packs logits CLASS-MAJOR per tile — [P, 10, w] bf16 with
class-slot order [2,3,4 | 0,1 | 5,6,7 | 8,9] so slot k pairs with slot k+5
inside one LUT group.  Every per-row reduction is then a tree of large
CONTIGUOUS [P, k*w] tensor_tensor ops, which hit the DVE's 2x bf16 packing
(~0.6 ns/col; tensor_reduce never packs, and GPSIMD is left idle since any
GPSIMD op halves 2-port DVE throughput through the shared SBUF port).

  DMA   : comb tile = [10w logits | w dup] bf16, 22 B/row, one DMA per tile.
          dup = pred[t] + 64*bt (bf16; bt threshold 32 has huge margin).
  ACT   : E = exp(logits) bf16; Ln(row-sum) with per-partition accumulate.
  DVE   : h5 = E[0:5w]+E[5w:10w]; t2/t1/s add-tree tail; mx = pairwise max;
          m6/m4 group maxes; d6 = m6-m4; dup accumulate (tensor_scalar,
          4x-packed bf16); COUNT_MM_ANT 2-port custom op:
          mismatch = bt ? (d6 < 0) : (d6 >= 0), accum -> count.
          argmax in exp space is exact (exp monotone); bf16 rounding ties
          are symmetric noise, well inside the 2e-2 budget.

Padded rows are all-zero logits (lse = ln 10, d6 = 0 -> counted once via
the bt=0 branch) and dup = 0; pad contributions and the 64*sum(bt) shift
are exact host-side constants.  Tile widths are EVEN so every class slice
is 4-byte aligned (bf16 2x mode requires it); ascending-then-small order
keeps the DMA stream ahead of compute and the drain tail short.
"""

import numpy as np

# ---------------------------------------------------------------- constants
N = 2_000_000
C = 10
N_CORES = 8
P = 128
TILE_WS = [60, 120, 280, 480, 608, 408]
W_SUM = sum(TILE_WS)          # 1,956
ROWS_CORE_PAD = P * W_SUM     # 250,368
ROWS_CORE = N // N_CORES      # 250,000
PAD_PER_CORE = ROWS_CORE_PAD - ROWS_CORE  # 368
# class-slot order: [g6a | g4a | g6b | g4b]; slot k pairs with slot k+5
# within the same LUT group: (2,5),(3,6),(4,7) in g6, (0,8),(1,9) in g4.
PERM = [2, 3, 4, 0, 1, 5, 6, 7, 8, 9]
BT_SHIFT = 64.0
BT_THRESH = 32.0

_CACHE = {}


# ------------------------------------------------------- custom DVE op
def _register_custom_ops():
    """Register the 2-port mismatch-count op (idempotent)."""
    import concourse.dve_ops as dve_ops
    from concourse.dve_spec import (
        Spec, Src0, Src1, Zero, select, lower, AluOp, Bin, C0,
    )
    from concourse.dve_uop import DveOpSpec

    def _get(name):
        for op in dve_ops.OPS:
            if op.name == name:
                return op
        return None

    def _register(name, spec):
        existing = _get(name)
        if existing is not None:
            return existing
        opcode = dve_ops._CUSTOM_DVE_ROW_BASE + len(dve_ops.OPS)
        assert opcode < 0x20, "custom DVE opcode rows exhausted"
        from concourse.dve_ops import has_src1
        shas = {}
        for ver in ("v3", "v4"):
            uops = lower(spec, ver=ver)
            tmp = DveOpSpec(name=name, opcode=opcode, uops=uops,
                            rd1_en=has_src1(spec))
            shas[ver] = tmp.sha(ver)
        op = dve_ops.DveOp(name, spec, subdim=False, uops_sha=shas)
        dve_ops.OPS.append(op)
        dve_ops._SUB_OPCODE_FOR_NAME[name] = opcode
        dve_ops.CUSTOM_DVE_SPECS[name] = spec
        return op

    # COUNT_MM: in0 = d6 = m6 - m4, in1 = dup (>= s0 encodes bt = 1).
    # mismatch = bt ? (d6 < 0) : (d6 >= 0); accum counts mismatches.
    def _count_ref(in0, in1, s0, s1, imm2):
        p = in0.shape[0]
        d6 = np.asarray(in0, np.float32).reshape(p, -1)
        dup = np.asarray(in1, np.float32).reshape(p, -1)
        s0v = np.asarray(s0, np.float32).reshape(p, 1) \
            if isinstance(s0, np.ndarray) else np.float32(s0)
        bt = dup >= s0v
        mm = np.where(bt, d6 < 0, d6 >= 0).astype(np.float32)
        acc = mm.sum(axis=1, dtype=np.float64).astype(np.float32)[:, None]
        return mm.reshape(in0.shape), acc

    body = select(Src1 >= C0,
                  Bin(AluOp.IS_LT, Src0, Zero),
                  Bin(AluOp.IS_GE, Src0, Zero))
    count_spec = Spec(
        body=body,
        accum=AluOp.ADD,
        accum_init=Zero,
        reference=_count_ref,
    )
    cop = _register("COUNT_MM_ANT", count_spec)
    return cop


# ------------------------------------------------------------- device build
def _build_nc(tile_ws=None):
    import concourse.bass as bass
    import concourse.tile as tile
    from concourse import bacc, mybir

    f32 = mybir.dt.float32
    bf16 = mybir.dt.bfloat16
    A = mybir.ActivationFunctionType
    X = mybir.AxisListType.X
    alu = mybir.AluOpType

    cop = _register_custom_ops()
    if tile_ws is None:
        tile_ws = TILE_WS
    tiles = len(tile_ws)
    nc = bacc.Bacc("TRN2", target_bir_lowering=False, debug=False,
                   num_devices=N_CORES)
    comb_ds = [
        nc.dram_tensor(f"comb{i}", [P, wi * (C + 1)], bf16,
                       kind="ExternalInput").ap()
        for i, wi in enumerate(tile_ws)
    ]
    out_d = nc.dram_tensor("out", [P, 3 * len(tile_ws if tile_ws else
                                               TILE_WS)], f32,
                           kind="ExternalOutput").ap()

    with tile.TileContext(nc) as tc:
        with (
            tc.tile_pool(name="io", bufs=6) as io,
            tc.tile_pool(name="ep", bufs=3) as ep,
            tc.tile_pool(name="mp", bufs=3) as mp,
            tc.tile_pool(name="cp", bufs=1) as cp,
        ):
            acc_all = cp.tile([P, 3, tiles], f32)
            acc_lg = acc_all[:, 0, :]
            acc_g = acc_all[:, 1, :]
            acc_mm = acc_all[:, 2, :]

            for i in range(tiles):
                w = tile_ws[i]
                ct = io.tile([P, w * (C + 1)], bf16, tag="comb")
                nc.sync.dma_start(ct[:], comb_ds[i])
                lt = ct[:, 0:w * C]
                dt = ct[:, w * C:w * (C + 1)]

                # ---- gathered-logit accumulate: big tiles go to ACT
                #      (Copy+accum) to balance engines, rest fill Vector's
                #      wait-for-EXP gap
                if w >= 400:
                    gj = mp.tile([P, w], bf16, tag="gj")
                    nc.scalar.activation(gj[:], dt, A.Copy,
                                         accum_out=acc_g[:, i:i + 1])
                else:
                    nc.vector.reduce_sum(acc_g[:, i:i + 1], dt, axis=X)

                # E = exp(logits), class-major [P, 10, w] flattened
                et = ep.tile([P, w * C], bf16, tag="E")
                nc.scalar.activation(et[:], lt, A.Exp)
                ev = et[:]

                # ---- row-sum tree (all DVE, contiguous bf16 2x)
                h5 = mp.tile([P, 5 * w], bf16, tag="h5")
                nc.vector.tensor_tensor(
                    h5[:], ev[:, 0:5 * w], ev[:, 5 * w:10 * w], op=alu.add)
                t2 = mp.tile([P, 2 * w], bf16, tag="t2")
                nc.vector.tensor_tensor(
                    t2[:], h5[:, 0:2 * w], h5[:, 2 * w:4 * w], op=alu.add)
                s = mp.tile([P, w], bf16, tag="s")
                nc.vector.tensor_tensor(
                    s[:], t2[:, 0:w], t2[:, w:2 * w], op=alu.add)
                nc.vector.tensor_tensor(
                    s[:], s[:], h5[:, 4 * w:5 * w], op=alu.add)

                lnj = mp.tile([P, w], bf16, tag="lnj")
                nc.scalar.activation(lnj[:], s[:], A.Ln,
                                     accum_out=acc_lg[:, i:i + 1])

                # ---- BCE: group max trees in exp space (DVE)
                mx = mp.tile([P, 5 * w], bf16, tag="mx")
                nc.vector.tensor_tensor(
                    mx[:], ev[:, 0:5 * w], ev[:, 5 * w:10 * w], op=alu.max)
                mxv = mx[:].rearrange("p (f w) -> p f w", w=w)
                u2 = mp.tile([P, 2, w], bf16, tag="u2")
                nc.vector.tensor_tensor(
                    u2[:], mxv[:, 0:4:3, :], mxv[:, 1:5:3, :], op=alu.max)
                m6 = mp.tile([P, w], bf16, tag="m6")
                nc.vector.tensor_tensor(
                    m6[:], u2[:, 0, :], mx[:, 2 * w:3 * w], op=alu.max)
                d6 = mp.tile([P, w], bf16, tag="d6")
                nc.vector.tensor_tensor(
                    d6[:], m6[:], u2[:, 1, :], op=alu.subtract)

                cj = mp.tile([P, w], bf16, tag="cj")
                nc.vector._custom_dve(
                    cop, out=cj[:], in0=d6[:], in1=dt, s0=BT_THRESH,
                    accum_out=acc_mm[:, i:i + 1])

            # ---- store raw per-tile partials; host folds them
            nc.sync.dma_start(out_d[:], acc_all[:])

    # Force a single activation table containing both Exp and Ln so the
    # compiler does not ping-pong ACT_TABLE_LOADs.  Table ids are positional,
    # so keep the dict shape and empty the other sets.
    import concourse.bacc as bacc_mod
    from concourse.hw_specs import get_activation_tables
    orig = get_activation_tables(nc.m.arch)
    combined = None
    for k, v in orig.items():
        if (mybir.ActivationFunctionType.Exp in v
                and mybir.ActivationFunctionType.Ln in v):
            combined = k
            break
    if combined is not None:
        patched = {k: (v if k == combined else set()) for k, v in orig.items()}
        saved = bacc_mod.get_activation_tables
        bacc_mod.get_activation_tables = lambda arch: patched
        try:
            nc.compile()
        finally:
            bacc_mod.get_activation_tables = saved
    else:
        nc.compile()
    return nc


def _get_nc():
    if "nc" not in _CACHE:
        _CACHE["nc"] = _build_nc()
    return _CACHE["nc"]


# ------------------------------------------------------------------- host
def _host_prep(pred, target):
    """Shard + pad inputs; pack bf16 class-major logits + bf16 dup rows."""
    from concourse import mybir
    bf16 = mybir.dt.np(mybir.dt.bfloat16)

    pred = np.asarray(pred, dtype=np.float32)
    target = np.asarray(target).astype(np.int64)

    in_maps = []
    rows = ROWS_CORE
    for c in range(N_CORES):
        pc = pred[c * rows:(c + 1) * rows]
        tc_ = target[c * rows:(c + 1) * rows]
        bt = (tc_ >= 2) & (tc_ <= 7)
        dup = (np.take_along_axis(pc, tc_[:, None], axis=1)[:, 0]
               + np.float32(BT_SHIFT) * bt).astype(bf16)
        lg = pc[:, PERM].astype(bf16)
        if PAD_PER_CORE:
            lg = np.concatenate(
                [lg, np.zeros((PAD_PER_CORE, C), bf16)], axis=0)
            dup = np.concatenate(
                [dup, np.zeros(PAD_PER_CORE, bf16)], axis=0)
        m = {}
        off = 0
        for i, wi in enumerate(TILE_WS):
            n_i = P * wi
            # class-major logits [P, 10, wi] then the dup column
            logi = lg[off:off + n_i].reshape(P, wi, C).transpose(
                0, 2, 1).reshape(P, wi * C)
            dupi = dup[off:off + n_i].reshape(P, wi)
            m[f"comb{i}"] = np.ascontiguousarray(
                np.concatenate([logi, dupi], axis=1))
            off += n_i
        in_maps.append(m)
    return in_maps


def kernel(pred, target):
    from concourse.bass_utils import run_bass_kernel_spmd

    nc = _get_nc()
    in_maps = _host_prep(pred, target)
    res = run_bass_kernel_spmd(nc, in_maps, core_ids=list(range(N_CORES)))

    target = np.asarray(target).astype(np.int64)
    nbt = int(((target >= 2) & (target <= 7)).sum())

    sum_lg = 0.0
    sum_g = 0.0
    sum_mm = 0.0
    tiles = len(TILE_WS)
    for c in range(N_CORES):
        o = res.results[c]["out"].astype(np.float64).reshape(P, 3, tiles)
        sum_lg += o[:, 0, :].sum()
        sum_g += o[:, 1, :].sum()
        sum_mm += o[:, 2, :].sum()

    # padded rows: logits = 0 -> lse = ln(10); d6 = 0 counts via the bt=0
    # branch; dup = 0 contributes nothing to sum_g.
    sum_lg -= N_CORES * PAD_PER_CORE * np.log(10.0)
    sum_g -= BT_SHIFT * nbt

    sum_mm -= N_CORES * PAD_PER_CORE
    ce = (sum_lg - sum_g) / N
    bce = 100.0 * sum_mm / N
    return np.float32(ce + bce)


# revision 31
# speedup vs baseline: 1.0625x; 1.0625x over previous
"""Trainium2 Bass kernel for nn_CustomLoss (CrossEntropy + binary-remap BCE).

loss = mean_i[ logsumexp(pred_i) - pred_i[t_i] ]
     + 100 * mean_i[ 1{ LUT[argmax(pred_i)] != LUT[t_i] } ]

with LUT = [0,0,1,1,1,1,1,1,0,0]  (LUT[j] = 1 iff 2 <= j <= 7).

Sharding: data-parallel over the batch axis across 8 NeuronCores; each core
returns per-partition partial sums which the host folds into the scalar.

Layout: the host packs logits CLASS-MAJOR per tile — [P, 10, w] bf16 with
class-slot order [2,3,4 | 0,1 | 5,6,7 | 8,9] so slot k pairs with slot k+5
inside one LUT group.  Every per-row reduction is then a tree of large
CONTIGUOUS [P, k*w] tensor_tensor ops, which hit the DVE's 2x bf16 packing
(~0.6 ns/col; tensor_reduce never packs, and GPSIMD is left idle since any
GPSIMD op halves 2-port DVE throughput through the shared SBUF port).

  DMA   : comb tile = [10w logits | w dup] bf16, 22 B/row, one DMA per tile.
          dup = pred[t] + 64*bt (bf16; bt threshold 32 has huge margin).
  ACT   : E = exp(logits) bf16; Ln(row-sum) with per-partition accumulate.
  DVE   : h5 = E[0:5w]+E[5w:10w]; t2/t1/s add-tree tail; mx = pairwise max;
          m6/m4 group maxes; d6 = m6-m4; dup accumulate (tensor_scalar,
          4x-packed bf16); COUNT_MM_ANT 2-port custom op:
          mismatch = bt ? (d6 < 0) : (d6 >= 0), accum -> count.
          argmax in exp space is exact (exp monotone); bf16 rounding ties
          are symmetric noise, well inside the 2e-2 budget.

Padded rows are all-zero logits (lse = ln 10, d6 = 0 -> counted once via
the bt=0 branch) and dup = 0; pad contributions and the 64*sum(bt) shift
are exact host-side constants.  Tile widths are EVEN so every class slice
is 4-byte aligned (bf16 2x mode requires it); ascending-then-small order
keeps the DMA stream ahead of compute and the drain tail short.
"""

import numpy as np

# ---------------------------------------------------------------- constants
N = 2_000_000
C = 10
N_CORES = 8
P = 128
TILE_WS = [60, 120, 200, 300, 420, 520, 336]
W_SUM = sum(TILE_WS)          # 1,956
ROWS_CORE_PAD = P * W_SUM     # 250,368
ROWS_CORE = N // N_CORES      # 250,000
PAD_PER_CORE = ROWS_CORE_PAD - ROWS_CORE  # 368
# class-slot order: [g6a | g4a | g6b | g4b]; slot k pairs with slot k+5
# within the same LUT group: (2,5),(3,6),(4,7) in g6, (0,8),(1,9) in g4.
PERM = [2, 3, 4, 0, 1, 5, 6, 7, 8, 9]
BT_SHIFT = 64.0
BT_THRESH = 32.0

_CACHE = {}


# ------------------------------------------------------- custom DVE op
def _register_custom_ops():
    """Register the 2-port mismatch-count op (idempotent)."""
    import concourse.dve_ops as dve_ops
    from concourse.dve_spec import (
        Spec, Src0, Src1, Zero, select, lower, AluOp, Bin, C0,
    )
    from concourse.dve_uop import DveOpSpec

    def _get(name):
        for op in dve_ops.OPS:
            if op.name == name:
                return op
        return None

    def _register(name, spec):
        existing = _get(name)
        if existing is not None:
            return existing
        opcode = dve_ops._CUSTOM_DVE_ROW_BASE + len(dve_ops.OPS)
        assert opcode < 0x20, "custom DVE opcode rows exhausted"
        from concourse.dve_ops import has_src1
        shas = {}
        for ver in ("v3", "v4"):
            uops = lower(spec, ver=ver)
            tmp = DveOpSpec(name=name, opcode=opcode, uops=uops,
                            rd1_en=has_src1(spec))
            shas[ver] = tmp.sha(ver)
        op = dve_ops.DveOp(name, spec, subdim=False, uops_sha=shas)
        dve_ops.OPS.append(op)
        dve_ops._SUB_OPCODE_FOR_NAME[name] = opcode
        dve_ops.CUSTOM_DVE_SPECS[name] = spec
        return op

    # COUNT_MM: in0 = d6 = m6 - m4, in1 = dup (>= s0 encodes bt = 1).
    # mismatch = bt ? (d6 < 0) : (d6 >= 0); accum counts mismatches.
    def _count_ref(in0, in1, s0, s1, imm2):
        p = in0.shape[0]
        d6 = np.asarray(in0, np.float32).reshape(p, -1)
        dup = np.asarray(in1, np.float32).reshape(p, -1)
        s0v = np.asarray(s0, np.float32).reshape(p, 1) \
            if isinstance(s0, np.ndarray) else np.float32(s0)
        bt = dup >= s0v
        mm = np.where(bt, d6 < 0, d6 >= 0).astype(np.float32)
        acc = mm.sum(axis=1, dtype=np.float64).astype(np.float32)[:, None]
        return mm.reshape(in0.shape), acc

    body = select(Src1 >= C0,
                  Bin(AluOp.IS_LT, Src0, Zero),
                  Bin(AluOp.IS_GE, Src0, Zero))
    count_spec = Spec(
        body=body,
        accum=AluOp.ADD,
        accum_init=Zero,
        reference=_count_ref,
    )
    cop = _register("COUNT_MM_ANT", count_spec)
    return cop


# ------------------------------------------------------------- device build
def _build_nc(tile_ws=None):
    import concourse.bass as bass
    import concourse.tile as tile
    from concourse import bacc, mybir

    f32 = mybir.dt.float32
    bf16 = mybir.dt.bfloat16
    A = mybir.ActivationFunctionType
    X = mybir.AxisListType.X
    alu = mybir.AluOpType

    cop = _register_custom_ops()
    if tile_ws is None:
        tile_ws = TILE_WS
    tiles = len(tile_ws)
    nc = bacc.Bacc("TRN2", target_bir_lowering=False, debug=False,
                   num_devices=N_CORES)
    comb_ds = [
        nc.dram_tensor(f"comb{i}", [P, wi * (C + 1)], bf16,
                       kind="ExternalInput").ap()
        for i, wi in enumerate(tile_ws)
    ]
    out_d = nc.dram_tensor("out", [P, 3 * len(tile_ws if tile_ws else
                                               TILE_WS)], f32,
                           kind="ExternalOutput").ap()

    with tile.TileContext(nc) as tc:
        with (
            tc.tile_pool(name="io", bufs=6) as io,
            tc.tile_pool(name="ep", bufs=3) as ep,
            tc.tile_pool(name="mp", bufs=3) as mp,
            tc.tile_pool(name="cp", bufs=1) as cp,
        ):
            acc_all = cp.tile([P, 3, tiles], f32)
            acc_lg = acc_all[:, 0, :]
            acc_g = acc_all[:, 1, :]
            acc_mm = acc_all[:, 2, :]

            for i in range(tiles):
                w = tile_ws[i]
                ct = io.tile([P, w * (C + 1)], bf16, tag="comb")
                nc.sync.dma_start(ct[:], comb_ds[i])
                lt = ct[:, 0:w * C]
                dt = ct[:, w * C:w * (C + 1)]

                # ---- gathered-logit accumulate (DVE reduce; runs while
                #      Vector waits on this tile's EXP)
                nc.vector.reduce_sum(acc_g[:, i:i + 1], dt, axis=X)

                # E = exp(logits), class-major [P, 10, w] flattened
                et = ep.tile([P, w * C], bf16, tag="E")
                nc.scalar.activation(et[:], lt, A.Exp)
                ev = et[:]

                # ---- row-sum tree (all DVE, contiguous bf16 2x)
                h5 = mp.tile([P, 5 * w], bf16, tag="h5")
                nc.vector.tensor_tensor(
                    h5[:], ev[:, 0:5 * w], ev[:, 5 * w:10 * w], op=alu.add)
                t2 = mp.tile([P, 2 * w], bf16, tag="t2")
                nc.vector.tensor_tensor(
                    t2[:], h5[:, 0:2 * w], h5[:, 2 * w:4 * w], op=alu.add)
                s = mp.tile([P, w], bf16, tag="s")
                nc.vector.tensor_tensor(
                    s[:], t2[:, 0:w], t2[:, w:2 * w], op=alu.add)
                nc.vector.tensor_tensor(
                    s[:], s[:], h5[:, 4 * w:5 * w], op=alu.add)

                lnj = mp.tile([P, w], bf16, tag="lnj")
                nc.scalar.activation(lnj[:], s[:], A.Ln,
                                     accum_out=acc_lg[:, i:i + 1])

                # ---- BCE: group max trees in exp space (DVE)
                mx = mp.tile([P, 5 * w], bf16, tag="mx")
                nc.vector.tensor_tensor(
                    mx[:], ev[:, 0:5 * w], ev[:, 5 * w:10 * w], op=alu.max)
                mxv = mx[:].rearrange("p (f w) -> p f w", w=w)
                u2 = mp.tile([P, 2, w], bf16, tag="u2")
                nc.vector.tensor_tensor(
                    u2[:], mxv[:, 0:4:3, :], mxv[:, 1:5:3, :], op=alu.max)
                m6 = mp.tile([P, w], bf16, tag="m6")
                nc.vector.tensor_tensor(
                    m6[:], u2[:, 0, :], mx[:, 2 * w:3 * w], op=alu.max)
                d6 = mp.tile([P, w], bf16, tag="d6")
                nc.vector.tensor_tensor(
                    d6[:], m6[:], u2[:, 1, :], op=alu.subtract)

                cj = mp.tile([P, w], bf16, tag="cj")
                nc.vector._custom_dve(
                    cop, out=cj[:], in0=d6[:], in1=dt, s0=BT_THRESH,
                    accum_out=acc_mm[:, i:i + 1])

            # ---- store raw per-tile partials; host folds them
            nc.sync.dma_start(out_d[:], acc_all[:])

    # Force a single activation table containing both Exp and Ln so the
    # compiler does not ping-pong ACT_TABLE_LOADs.  Table ids are positional,
    # so keep the dict shape and empty the other sets.
    import concourse.bacc as bacc_mod
    from concourse.hw_specs import get_activation_tables
    orig = get_activation_tables(nc.m.arch)
    combined = None
    for k, v in orig.items():
        if (mybir.ActivationFunctionType.Exp in v
                and mybir.ActivationFunctionType.Ln in v):
            combined = k
            break
    if combined is not None:
        patched = {k: (v if k == combined else set()) for k, v in orig.items()}
        saved = bacc_mod.get_activation_tables
        bacc_mod.get_activation_tables = lambda arch: patched
        try:
            nc.compile()
        finally:
            bacc_mod.get_activation_tables = saved
    else:
        nc.compile()
    return nc


def _get_nc():
    if "nc" not in _CACHE:
        _CACHE["nc"] = _build_nc()
    return _CACHE["nc"]


# ------------------------------------------------------------------- host
def _host_prep(pred, target):
    """Shard + pad inputs; pack bf16 class-major logits + bf16 dup rows."""
    from concourse import mybir
    bf16 = mybir.dt.np(mybir.dt.bfloat16)

    pred = np.asarray(pred, dtype=np.float32)
    target = np.asarray(target).astype(np.int64)

    in_maps = []
    rows = ROWS_CORE
    for c in range(N_CORES):
        pc = pred[c * rows:(c + 1) * rows]
        tc_ = target[c * rows:(c + 1) * rows]
        bt = (tc_ >= 2) & (tc_ <= 7)
        dup = (np.take_along_axis(pc, tc_[:, None], axis=1)[:, 0]
               + np.float32(BT_SHIFT) * bt).astype(bf16)
        lg = pc[:, PERM].astype(bf16)
        if PAD_PER_CORE:
            lg = np.concatenate(
                [lg, np.zeros((PAD_PER_CORE, C), bf16)], axis=0)
            dup = np.concatenate(
                [dup, np.zeros(PAD_PER_CORE, bf16)], axis=0)
        m = {}
        off = 0
        for i, wi in enumerate(TILE_WS):
            n_i = P * wi
            # class-major logits [P, 10, wi] then the dup column
            logi = lg[off:off + n_i].reshape(P, wi, C).transpose(
                0, 2, 1).reshape(P, wi * C)
            dupi = dup[off:off + n_i].reshape(P, wi)
            m[f"comb{i}"] = np.ascontiguousarray(
                np.concatenate([logi, dupi], axis=1))
            off += n_i
        in_maps.append(m)
    return in_maps


def kernel(pred, target):
    from concourse.bass_utils import run_bass_kernel_spmd

    nc = _get_nc()
    in_maps = _host_prep(pred, target)
    res = run_bass_kernel_spmd(nc, in_maps, core_ids=list(range(N_CORES)))

    target = np.asarray(target).astype(np.int64)
    nbt = int(((target >= 2) & (target <= 7)).sum())

    sum_lg = 0.0
    sum_g = 0.0
    sum_mm = 0.0
    tiles = len(TILE_WS)
    for c in range(N_CORES):
        o = res.results[c]["out"].astype(np.float64).reshape(P, 3, tiles)
        sum_lg += o[:, 0, :].sum()
        sum_g += o[:, 1, :].sum()
        sum_mm += o[:, 2, :].sum()

    # padded rows: logits = 0 -> lse = ln(10); d6 = 0 counts via the bt=0
    # branch; dup = 0 contributes nothing to sum_g.
    sum_lg -= N_CORES * PAD_PER_CORE * np.log(10.0)
    sum_g -= BT_SHIFT * nbt

    sum_mm -= N_CORES * PAD_PER_CORE
    ce = (sum_lg - sum_g) / N
    bce = 100.0 * sum_mm / N
    return np.float32(ce + bce)
